# revision 10
# baseline (speedup 1.0000x reference)
"""Conv2d 3x3 (stride 1, pad 1, cross-correlation) + scalar bias on 8 TRN2 cores.

Full inputs:  x (32, 128, 56, 56) f32, K (256, 128, 3, 3) f32, bias (1,) f32
Full output:  (32, 256, 56, 56) f32

Sharding: data-parallel over the batch dim — each of the 8 NeuronCores gets 4
images; K and bias are replicated. No collectives needed.

Per-core algorithm (implicit GEMM via shifted matmuls):
  - Host zero-pads each image to 58x58 and lays it out as [Cin=128, 58*58]
    (Cin on SBUF partitions = the matmul contraction dim).
  - For each output row-tile of 8 padded rows (8*58 = 464 moving elements) and
    each Cout chunk of 128, accumulate 9 matmuls in one PSUM bank:
        out[co, p] += K[co, ci, dy, dx] * xpad[ci, p + (dy-1)*58 + (dx-1)]
    lhsT = K slice [ci=128, co=128] (stationary), rhs = shifted xpad slice.
  - Operands are float32r: fp32 bits in memory, PE runs them at full (bf16)
    rate for moving dims >= 256 (walrus requires lhsT/rhs dtypes to match).
  - Input images are loaded in overlapping 10-row halo chunks so the PE can
    start after ~2 chunks instead of after the whole 7 MB input load.
  - PSUM is evacuated through ScalarE activation(Identity, bias=...) which
    folds in the scalar bias, then DMA'd to HBM in a padded 58-wide layout;
    the host strips the 2 padding columns.
"""

import numpy as np

import concourse.tile as tile
import concourse.mybir as mybir
from concourse import bacc
from concourse import bass_utils

N, CIN, H, W = 32, 128, 56, 56
COUT, KH, KW = 256, 3, 3
NCORES = 8
B = N // NCORES            # images per core
HP, WP = H + 2, W + 2      # zero-padded image dims (58x58)
IMG = HP * WP              # 3364
XLEN = IMG + 4             # lead margin + tail slack so the 8x58 rearrange
                           # window of the last row-tile stays in-bounds
ROWS = 8                   # output rows per PSUM tile
NT = ROWS * W              # 448 moving elements per matmul (8 rows x 56 cols,
                           # strided 58 via a 2D AP -- pad cols not computed)
NRT = H // ROWS            # 7 row-tiles per image
OW = H * W                 # dense output block per (n, co): 56*56
GROUP = 4                  # row-tiles evacuated as a batch

F32 = mybir.dt.float32
F32R = mybir.dt.float32r
BF16 = mybir.dt.bfloat16

MM_DTYPE = BF16            # matmul operand dtype (walrus requires lhsT == rhs)
OUT_DTYPE = BF16           # SBUF/HBM output dtype; host converts back to f32

_CACHE = {}


def _build(nreps=1, mm_dtype=None):
    mm_dtype = MM_DTYPE if mm_dtype is None else mm_dtype
    nc = bacc.Bacc("TRN2", target_bir_lowering=False, debug=False)

    x_d = [
        nc.dram_tensor(f"x{n}", [CIN, XLEN], mm_dtype, kind="ExternalInput")
        for n in range(B)
    ]
    kw_d = nc.dram_tensor("kw", [CIN, KH * KW * COUT], mm_dtype, kind="ExternalInput")
    b_d = nc.dram_tensor("biasv", [CIN, 1], F32, kind="ExternalInput")
    y_d = nc.dram_tensor("y", [B, COUT, OW], OUT_DTYPE, kind="ExternalOutput")

    with tile.TileContext(nc) as tc:
        rep_ctx = tc.For_i(0, nreps, 1) if nreps > 1 else None
        if rep_ctx is not None:
            rep_ctx.__enter__()
        with (
            tc.tile_pool(name="const", bufs=1) as const,
            tc.tile_pool(name="psum", bufs=8, space="PSUM") as psum,
            tc.tile_pool(name="outs", bufs=8) as outs,
        ):
            # PE pre-warm: dummy matmuls on scratch (uninitialized) SBUF while
            # the first input DMAs are in flight, so HAM reaches full clock
            # before the first real matmul.
            wsrc = const.tile([CIN, 640], mm_dtype, tag="warm_src")
            nc.vector.memset(wsrc[:].bitcast(F32), 0.0)
            warm = psum.tile([128, 512], F32, name="warm", tag="pt")
            for _ in range(6):
                nc.tensor.matmul(
                    warm[:], wsrc[:, :128], wsrc[:, 128:640], start=True, stop=True
                )

            # Weights, laid out host-side as [ci, chunk, dydx, co128] so each
            # Cout-chunk half is one contiguous DMA on the scalar HWDGE queue
            # (chunk 0 first — it gates the first matmuls).
            # One SBUF tile per image, filled by disjoint chunk DMAs in
            # consumption order; Tile's subtile dependency tracking lets
            # row-tile i start once its covering chunks have landed. The
            # first chunk is exactly the rows row-tile 0 needs (0..9) so the
            # first matmul can start as early as possible; weights load in
            # parallel on the scalar queue.
            CUTS = [0, 1 + 10 * WP, 1 + 26 * WP, 1 + 42 * WP, XLEN]
            xin = [
                const.tile([CIN, XLEN], mm_dtype, name="xt", tag=f"x{n}")
                for n in range(B)
            ]
            nc.sync.dma_start(xin[0][:, : CUTS[1]], x_d[0][:, : CUTS[1]])

            kw = const.tile([CIN, KH * KW * COUT], mm_dtype, tag="kw")
            half = KH * KW * 128  # 1152
            # First position's weights as a tiny DMA so the first matmul can
            # start immediately; the rest in two bulk transfers.
            nc.scalar.dma_start(kw[:, 0:128], kw_d[:, 0:128])
            nc.scalar.dma_start(kw[:, 128:half], kw_d[:, 128:half])
            nc.scalar.dma_start(kw[:, half : 2 * half], kw_d[:, half : 2 * half])
            bias = const.tile([CIN, 1], F32, tag="bias")
            nc.gpsimd.dma_start(bias[:], b_d[:])

            for n in range(B):
                for c in range(4):
                    if n == 0 and c == 0:
                        continue
                    lo, hi = CUTS[c], CUTS[c + 1]
                    nc.sync.dma_start(xin[n][:, lo:hi], x_d[n][:, lo:hi])

            def evacuate(pt, chunk, n, i, use_act, final=False):
                # Split PSUM evacuation across ScalarE and VectorE so bank
                # release (and the kernel tail) isn't serialized on one
                # engine. Both fold in the scalar bias.
                ot = outs.tile([128, NT], OUT_DTYPE, name="ot", tag="ot")

                def evac_slice(sl, on_act):
                    if on_act:
                        nc.scalar.activation(
                            ot[:, sl],
                            pt[:, sl],
                            mybir.ActivationFunctionType.Identity,
                            bias=bias[:],
                        )
                    else:
                        nc.vector.tensor_scalar_add(ot[:, sl], pt[:, sl], bias[:])

                out_eng = nc.scalar if chunk == 0 else nc.sync
                ydst = y_d[
                    n,
                    chunk * 128 : (chunk + 1) * 128,
                    i * NT : (i + 1) * NT,
                ]
                if final:
                    # tail: halved evac+DMA, the two halves on different
                    # engines and DGE queues so they drain in parallel
                    hn = NT // 2
                    evac_slice(slice(0, hn), on_act=False)
                    nc.sync.dma_start(ydst[:, :hn], ot[:, :hn])
                    evac_slice(slice(hn, NT), on_act=True)
                    nc.scalar.dma_start(ydst[:, hn:], ot[:, hn:])
                else:
                    evac_slice(slice(0, NT), on_act=use_act)
                    out_eng.dma_start(ydst, ot[:])

            def mm(pt, chunk, n, i, dy, dx, ki):
                wlo = chunk * half + (dy * 3 + dx) * 128
                w = kw[:, wlo : wlo + 128]
                # rhs for output rows 8i..8i+7 (padded rows 8i+1..8i+8), tap
                # (dy, dx): 8 rows of 56 valid cols, row stride WP=58.
                # Image data starts at element 1 of the per-image tile.
                base = 1 + (8 * i + dy) * WP + dx
                rhs = xin[n][:, base : base + ROWS * WP].rearrange(
                    "p (r c) -> p r c", r=ROWS, c=WP
                )[:, :, :W]
                nc.tensor.matmul(pt[:], w, rhs, start=(ki == 0), stop=(ki == 8))

            tiles = [(n, i) for n in range(B) for i in range(NRT)]  # 28 row-tiles
            for g in range(0, len(tiles), GROUP):
                grp = tiles[g : g + GROUP]
                last_group = g + GROUP >= len(tiles)
                for chunk in range(2):
                    pts = [
                        psum.tile([128, NT], F32, name="pt", tag="pt") for _ in grp
                    ]
                    if last_group:
                        # Tail: tile-major so each tile's accumulation closes
                        # early and its evacuation+DMA overlaps the next
                        # tile's matmuls; alternate evacuation engines.
                        for t, (n, i) in enumerate(grp):
                            for ki, (dy, dx) in enumerate(
                                (dy, dx) for dy in range(3) for dx in range(3)
                            ):
                                mm(pts[t], chunk, n, i, dy, dx, ki)
                            evacuate(
                                pts[t], chunk, n, i,
                                use_act=(t % 2 == 0),
                                final=(chunk == 1 and t == len(grp) - 1),
                            )
                    else:
                        # Steady state: dydx-major so 4 consecutive matmuls
                        # share the same stationary weights.
                        for ki, (dy, dx) in enumerate(
                            (dy, dx) for dy in range(3) for dx in range(3)
                        ):
                            for t, (n, i) in enumerate(grp):
                                mm(pts[t], chunk, n, i, dy, dx, ki)
                        for t, (n, i) in enumerate(grp):
                            evacuate(pts[t], chunk, n, i, use_act=(chunk == 0))
        if rep_ctx is not None:
            rep_ctx.__exit__(None, None, None)

    nc.compile()
    return nc


def _get_nc():
    if "nc" not in _CACHE:
        _CACHE["nc"] = _build()
    return _CACHE["nc"]


def _prep_in_maps(x, K, bias, mm_dtype=None):
    mm_dtype = MM_DTYPE if mm_dtype is None else mm_dtype
    np_dt = mybir.dt.np(mm_dtype)
    x = np.ascontiguousarray(x, dtype=np.float32)
    K = np.ascontiguousarray(K, dtype=np.float32)
    bias = np.asarray(bias, dtype=np.float32)

    # kw[ci, chunk*1152 + (dy*3+dx)*128 + co128] = K[chunk*128 + co128, ci, dy, dx]
    kw = (
        K.transpose(1, 2, 3, 0)                    # (ci, dy, dx, co)
        .reshape(CIN, KH * KW, 2, 128)             # split co -> (chunk, co128)
        .transpose(0, 2, 1, 3)                     # (ci, chunk, dydx, co128)
        .reshape(CIN, KH * KW * COUT)
        .astype(np_dt)
    )
    kw = np.ascontiguousarray(kw)
    biasv = np.full((CIN, 1), bias.reshape(-1)[0], dtype=np.float32)

    # Per-core padded inputs: [CIN, 1 + 58*58 + 1] with zero borders/margins.
    xbuf = np.zeros((NCORES, B, CIN, XLEN), dtype=np_dt)
    view = xbuf[:, :, :, 1 : 1 + IMG].reshape(NCORES, B, CIN, HP, WP)
    view[:, :, :, 1 : 1 + H, 1 : 1 + W] = x.reshape(NCORES, B, CIN, H, W).astype(np_dt)

    in_maps = []
    for c in range(NCORES):
        m = {"kw": kw, "biasv": biasv}
        for n in range(B):
            m[f"x{n}"] = np.ascontiguousarray(xbuf[c, n])
        in_maps.append(m)
    return in_maps


def run_on_cores(x, K, bias, trace=False):
    """Run the SPMD kernel; returns (full_output, BassKernelResults)."""
    nc = _get_nc()
    in_maps = _prep_in_maps(x, K, bias)
    res = bass_utils.run_bass_kernel_spmd(
        nc, in_maps, core_ids=list(range(NCORES)), trace=trace
    )
    out = np.empty((N, COUT, H, W), dtype=np.float32)
    for c in range(NCORES):
        yc = res.results[c]["y"].reshape(B, COUT, H, W)
        out[c * B : (c + 1) * B] = yc.astype(np.float32)
    return out, res


def kernel(x, K, bias):
    out, _ = run_on_cores(x, K, bias, trace=False)
    return out



# revision 16
# speedup vs baseline: 1.3281x; 1.3281x over previous
"""Conv2d 3x3 via 1D Winograd F(2,3) along rows + direct dx taps, 8 TRN2 cores.

Full inputs:  x (32, 128, 56, 56) f32, K (256, 128, 3, 3) f32, bias (1,) f32
Full output:  (32, 256, 56, 56) f32

Sharding: data-parallel over batch (4 images/core), K and bias replicated.

Per-core algorithm:
  - Host zero-pads each image to 58x58, casts to bf16, lays out [Cin, 58*58].
  - DVE builds 4 Winograd row-transform planes per image (ty = output row
    pairs, 28 per image):
        V0[ty] = d[2ty]   - d[2ty+2]
        V1[ty] = d[2ty+1] + d[2ty+2]
        V2[ty] = d[2ty+2] - d[2ty+1]
        V3[ty] = d[2ty+1] - d[2ty+3]
    (d[r] = padded input row r). Each plane is [Cin, 28*58] bf16.
  - Host pre-transforms weights: Kt[p] = sum_ky G[p,ky] K[:,:,ky,:] with
    G = [[1,0,0],[.5,.5,.5],[.5,-.5,.5],[0,0,1]] (exact in f32, cast bf16).
  - PE: for each (img, st-tile of 4 ty, cout chunk, p, dx) accumulate 3 dx
    taps into PSUM bank M[p] (232 moving cols per matmul, contiguous rhs).
    12 matmul-columns per 2 output rows instead of direct conv's 18.
  - DVE output transform per tile pair reads the 4 M banks:
        u = M1 + M2;  out[2ty]   = (M0 + bias) + u
        v = M1 - M2;  out[2ty+1] = (v  + bias) - M3
    writing row-interleaved bf16 SBUF tiles, DMA'd to a padded 58-wide
    layout; host strips the 2 pad columns and casts back to f32.
"""

import numpy as np

import concourse.tile as tile
import concourse.mybir as mybir
from concourse import bacc
from concourse import bass_utils

N, CIN, H, W = 32, 128, 56, 56
COUT, KH, KW = 256, 3, 3
NCORES = 8
B = N // NCORES            # images per core
HP, WP = H + 2, W + 2      # padded image dims (58x58)
IMG = HP * WP              # 3364
XLEN = IMG + 2 + WP        # +1 lead margin, tail slack for stride-2 row views
TY = H // 2                # 28 output row-pairs per image
P4 = 4                     # Winograd transform points
VLEN = TY * WP + 2         # 1626: V plane + 1-elem lead/tail margins
STY = 4                    # ty per matmul tile
NST = TY // STY            # 7 tiles per image
STN = STY * WP             # 232 moving cols per matmul
ORT = 2 * STY * WP         # 464 output cols (8 rows x 58) per tile
OWPAD = H * WP             # padded output block per (n, co): 56 rows * 58

F32 = mybir.dt.float32
BF16 = mybir.dt.bfloat16
AL = mybir.AluOpType

# tile groups per image: tiles in one group share LDWEIGHTS; each group uses
# one PSUM bank set (pairs of 232-wide tiles per [128, 464] bank, 4 points)
GROUPS = [(0, 1, 2, 3), (4, 5, 6)]

# input DMA chunks (row ranges); group g's V build needs padded rows
# 16*g .. 16*g+17 (+a up to 3)
ROWCUT = [0, 19, 38, HP]

_CACHE = {}


def _build(nreps=1):
    nc = bacc.Bacc("TRN2", target_bir_lowering=False, debug=False)

    x_d = [
        nc.dram_tensor(f"x{n}", [CIN, XLEN], BF16, kind="ExternalInput")
        for n in range(B)
    ]
    kw_d = nc.dram_tensor("kw", [CIN, 2 * P4 * 3 * 128], BF16, kind="ExternalInput")
    b_d = nc.dram_tensor("biasv", [CIN, 1], F32, kind="ExternalInput")
    y_d = nc.dram_tensor("y", [B, COUT, OWPAD], BF16, kind="ExternalOutput")

    with tile.TileContext(nc) as tc:
        rep_ctx = tc.For_i(0, nreps, 1) if nreps > 1 else None
        if rep_ctx is not None:
            rep_ctx.__enter__()
        with (
            tc.tile_pool(name="const", bufs=1) as const,
            tc.tile_pool(name="psum", bufs=8, space="PSUM") as psum,
            tc.tile_pool(name="outs", bufs=6) as outs,
            tc.tile_pool(name="scr", bufs=10) as scr,
        ):
            # PE pre-warm while the first DMAs land
            wsrc = const.tile([CIN, 640], BF16, tag="warm_src")
            nc.vector.memset(wsrc[:].bitcast(F32), 0.0)
            warm = psum.tile([128, 512], F32, name="warm", tag="pt")
            for _ in range(6):
                nc.tensor.matmul(
                    warm[:], wsrc[:, :128], wsrc[:, 128:640], start=True, stop=True
                )

            # input images, chunked by row ranges in consumption order
            xin = [
                const.tile([CIN, XLEN], BF16, name="xt", tag=f"x{n}")
                for n in range(B)
            ]
            lo0, hi0 = 0, 1 + ROWCUT[1] * WP
            nc.sync.dma_start(xin[0][:, lo0:hi0], x_d[0][:, lo0:hi0])

            kw = const.tile([CIN, 2 * P4 * 3 * 128], BF16, tag="kw")
            # first (ch0, p0) weights small DMA, then the rest
            nc.scalar.dma_start(kw[:, 0:384], kw_d[:, 0:384])
            nc.scalar.dma_start(kw[:, 384:3072], kw_d[:, 384:3072])
            bias = const.tile([CIN, 1], F32, tag="bias")
            nc.gpsimd.dma_start(bias[:], b_d[:])

            for n in range(B):
                for c in range(3):
                    if n == 0 and c == 0:
                        continue
                    lo = 1 + ROWCUT[c] * WP if c else 0
                    hi = 1 + ROWCUT[c + 1] * WP if c < 2 else XLEN
                    nc.sync.dma_start(xin[n][:, lo:hi], x_d[n][:, lo:hi])

            # Winograd V planes, one per (img, point)
            vpl = [
                [
                    const.tile([CIN, VLEN], BF16, name="vt", tag=f"v{n}_{p}")
                    for p in range(P4)
                ]
                for n in range(B)
            ]
            for n in range(B):
                for p in range(P4):
                    nc.vector.memset(vpl[n][p][:, 0:1], 0.0)
                    nc.vector.memset(vpl[n][p][:, VLEN - 1 : VLEN], 0.0)

            def drow(n, ty0, rows, a):
                # [128, rows, 58] view of padded rows 2*ty+a, ty = ty0..ty0+rows-1
                off = 1 + (2 * ty0 + a) * WP
                return (
                    xin[n][:, off : off + 2 * rows * WP]
                    .rearrange("c (t k w) -> c t k w", t=rows, k=2, w=WP)[:, :, 0, :]
                )

            def vbuild(n, g):
                # V planes for ty range of group g (8 or 4 ty)
                sts = GROUPS[g]
                ty0 = sts[0] * STY
                rows = STY * len(sts)
                spec = [(0, 2, AL.subtract), (1, 2, AL.add), (2, 1, AL.subtract),
                        (1, 3, AL.subtract)]
                for p, (a0, a1, op) in enumerate(spec):
                    dst = (
                        vpl[n][p][:, 1 + ty0 * WP : 1 + (ty0 + rows) * WP]
                        .rearrange("c (t w) -> c t w", t=rows, w=WP)
                    )
                    nc.vector.tensor_tensor(
                        dst, drow(n, ty0, rows, a0), drow(n, ty0, rows, a1), op
                    )

            for n in range(B):
                for g in range(len(GROUPS)):
                    vbuild(n, g)

            def wslice(ch, p, dx):
                o = ((ch * P4 + p) * 3 + dx) * 128
                return kw[:, o : o + 128]

            for n in range(B):
                for sts in GROUPS:
                    pairs = [sts[i : i + 2] for i in range(0, len(sts), 2)]
                    for ch in range(2):
                        m = [
                            [
                                psum.tile(
                                    [128, STN * len(pr)], F32, name="pt", tag="pt"
                                )
                                for p in range(P4)
                            ]
                            for pr in pairs
                        ]
                        for p in range(P4):
                            for dx in range(3):
                                for si, st in enumerate(sts):
                                    pi, j = si // 2, si % 2
                                    base = 1 + st * STN + dx - 1
                                    # start clears the whole bank's has_written
                                    # bits -> exactly one start/stop per bank
                                    nc.tensor.matmul(
                                        m[pi][p][:, j * STN : (j + 1) * STN],
                                        wslice(ch, p, dx),
                                        vpl[n][p][:, base : base + STN],
                                        start=(dx == 0 and j == 0),
                                        stop=(dx == 2 and j == len(pairs[pi]) - 1),
                                    )
                        # output transform: ACT stages, DVE combines, GPSIMD
                        # takes the SBUF-only final add
                        for pi, pr in enumerate(pairs):
                            width = STN * len(pr)
                            owidth = 2 * width
                            nty = STY * len(pr)
                            m0, m1, m2, m3 = (m[pi][p] for p in range(P4))

                            def view(t):
                                return t[:].rearrange(
                                    "c (t w) -> c t w", t=nty, w=WP
                                )

                            c2 = scr.tile([128, width], F32, name="c2", tag="scr")
                            nc.scalar.copy(c2[:], m2[:])
                            t3 = scr.tile([128, width], F32, name="t3", tag="scr")
                            nc.scalar.activation(
                                t3[:],
                                m3[:],
                                mybir.ActivationFunctionType.Identity,
                                bias=bias[:],
                                scale=-1.0,
                            )
                            u = scr.tile([128, width], F32, name="u", tag="scr")
                            nc.vector.tensor_tensor(u[:], m1[:], c2[:], AL.add)
                            v = scr.tile([128, width], F32, name="v", tag="scr")
                            nc.vector.tensor_tensor(v[:], m1[:], c2[:], AL.subtract)
                            o = outs.tile([128, owidth], BF16, name="ot", tag="ot")
                            o0 = o[:].rearrange(
                                "c (t k w) -> c t k w", t=nty, k=2, w=WP
                            )[:, :, 0, :]
                            o1 = o[:].rearrange(
                                "c (t k w) -> c t k w", t=nty, k=2, w=WP
                            )[:, :, 1, :]
                            nc.vector.scalar_tensor_tensor(
                                o0, view(m0), bias[:], view(u), AL.add, AL.add
                            )
                            nc.gpsimd.tensor_add(o1, view(v), view(t3))
                            out_eng = nc.scalar if ch == 0 else nc.sync
                            out_eng.dma_start(
                                y_d[
                                    n,
                                    ch * 128 : (ch + 1) * 128,
                                    pr[0] * ORT : pr[0] * ORT + owidth,
                                ],
                                o[:],
                            )
        if rep_ctx is not None:
            rep_ctx.__exit__(None, None, None)

    nc.compile()
    return nc


def _get_nc():
    if "nc" not in _CACHE:
        _CACHE["nc"] = _build()
    return _CACHE["nc"]


def _prep_in_maps(x, K, bias):
    np_bf16 = mybir.dt.np(BF16)
    x = np.ascontiguousarray(x, dtype=np.float32)
    K = np.ascontiguousarray(K, dtype=np.float32)
    bias = np.asarray(bias, dtype=np.float32)

    G = np.array(
        [[1, 0, 0], [0.5, 0.5, 0.5], [0.5, -0.5, 0.5], [0, 0, 1]], dtype=np.float32
    )
    # Kt[p, co, ci, dx]
    Kt = np.einsum("pk,oikx->poix", G, K)
    # kw[ci, ch, p, dx, co128]
    kw = (
        Kt.reshape(P4, 2, 128, CIN, 3)
        .transpose(3, 1, 0, 4, 2)
        .reshape(CIN, 2 * P4 * 3 * 128)
        .astype(np_bf16)
    )
    kw = np.ascontiguousarray(kw)
    biasv = np.full((CIN, 1), bias.reshape(-1)[0], dtype=np.float32)

    xbuf = np.zeros((NCORES, B, CIN, XLEN), dtype=np_bf16)
    view = xbuf[:, :, :, 1 : 1 + IMG].reshape(NCORES, B, CIN, HP, WP)
    view[:, :, :, 1 : 1 + H, 1 : 1 + W] = x.reshape(NCORES, B, CIN, H, W).astype(
        np_bf16
    )

    in_maps = []
    for c in range(NCORES):
        m = {"kw": kw, "biasv": biasv}
        for n in range(B):
            m[f"x{n}"] = np.ascontiguousarray(xbuf[c, n])
        in_maps.append(m)
    return in_maps


def run_on_cores(x, K, bias, trace=False):
    nc = _get_nc()
    in_maps = _prep_in_maps(x, K, bias)
    res = bass_utils.run_bass_kernel_spmd(
        nc, in_maps, core_ids=list(range(NCORES)), trace=trace
    )
    out = np.empty((N, COUT, H, W), dtype=np.float32)
    for c in range(NCORES):
        ypad = res.results[c]["y"].reshape(B, COUT, H, WP)
        out[c * B : (c + 1) * B] = ypad[:, :, :, 1 : 1 + W].astype(np.float32)
    return out, res


def kernel(x, K, bias):
    out, _ = run_on_cores(x, K, bias, trace=False)
    return out
